# revision 1
# baseline (speedup 1.0000x reference)
"""Multi-head attention (B=2, D=1024, L=2048, H=16) on 8 TRN2 NeuronCores.

Sharding: core c handles batch c//4 and query block c%4 (512 queries).
Each core computes K/V projections for its whole batch (duplicated across
the 4 cores sharing a batch -- this avoids any inter-core collective),
attention for its 512 queries over all 16 heads, and the output
projection for its query slice.  Host concatenates the 8 (1024, 512)
slices into the (2, 1024, 2048) output.

Layout choices (per core):
  - Scores are computed transposed: ST[k, q] = sum_d K[d,k] Q[d,q] with
    Lk on partitions, so exp(ST) tiles feed the A@V matmul as the moving
    operand with Lk as the contraction dim.
  - V is produced directly in transposed layout V^T (Lk x DH) by the
    projection out = x_chunk.T @ WvT_chunk, with a ones-column appended
    per head so the A@V matmul also emits the softmax denominator row.
  - Normalization is deferred: unnormalized C and all 16 denominator
    rows are stashed, then one (16, 512) reciprocal + 8 fp32 selector
    matmuls broadcast 1/denom across partitions, one multiply per
    128-row block.  Keeps multi-us serial work off the per-head path so
    the PE never idles long enough for the HAM clock gate to re-throttle.

All matmuls in bf16 (f32 PSUM accumulate); softmax stats in f32.
"""

import sys
import types

import numpy as np
import ml_dtypes


def _install_axon_hooks_shim():
    """antenv.axon_hooks is absent in this image; concourse imports it when
    tracing is requested (e.g. via the BASS_TRACE env var).  Provide the
    module and, if possible, the real NTFF profiling hook so tracing works
    instead of crashing."""
    try:
        import antenv.axon_hooks  # noqa: F401
        return
    except ImportError:
        pass
    try:
        import antenv
    except ImportError:
        return
    mod = types.ModuleType("antenv.axon_hooks")
    mod._hook = None
    mod.set_axon_ntff_profile_hook = lambda h: setattr(mod, "_hook", h)
    mod.get_axon_ntff_profile_hook = lambda: mod._hook
    sys.modules["antenv.axon_hooks"] = mod
    antenv.axon_hooks = mod
    try:
        from trn_agent_boot.trn_boot import _ntff_profile_via_ctypes

        h = _ntff_profile_via_ctypes("/opt/axon/libaxon_pjrt.so")
        if h is not None:
            mod._hook = h
    except Exception:
        pass


_install_axon_hooks_shim()

import concourse.bass as bass
import concourse.mybir as mybir
import concourse.tile as tile
from concourse import bacc
from concourse.bass_utils import run_bass_kernel_spmd
from concourse.tile_rust import add_dep_helper

BF16 = mybir.dt.bfloat16
F32 = mybir.dt.float32
AF = mybir.ActivationFunctionType

B, D, L, H = 2, 1024, 2048, 16
DH = D // H            # 64
P = 128
LQ = L // 4            # 512 queries per core
SCALE = 1.0 / np.sqrt(np.float32(DH))

DC = D // P            # 8 contraction chunks
LT = L // P            # 16 Lk tiles
HV = DH + 1            # V^T per-head width incl. ones column


def build():
    nc = bacc.Bacc(None, target_bir_lowering=False, debug=False)

    x = nc.dram_tensor("x", [D, L], BF16, kind="ExternalInput")
    xq = nc.dram_tensor("xq", [D, LQ], BF16, kind="ExternalInput")
    wqt = nc.dram_tensor("wqt", [D, D], BF16, kind="ExternalInput")
    wkt = nc.dram_tensor("wkt", [D, D], BF16, kind="ExternalInput")
    wvt = nc.dram_tensor("wvt", [D, D], BF16, kind="ExternalInput")
    wot = nc.dram_tensor("wot", [D, D], BF16, kind="ExternalInput")
    selp = nc.dram_tensor("selp", [2, P], F32, kind="ExternalInput")
    out = nc.dram_tensor("out", [D, LQ], F32, kind="ExternalOutput")

    xr = x[:].rearrange("(o p) l -> p o l", p=P)        # (128, 8, 2048)
    xqr = xq[:].rearrange("(o p) l -> p o l", p=P)      # (128, 8, 512)
    wqr = wqt[:].rearrange("(ko kp) o -> kp ko o", kp=P)  # (128, 8, 1024)
    wkr = wkt[:].rearrange("(ko kp) o -> kp ko o", kp=P)
    wvr = wvt[:].rearrange("(ko kp) o -> kp ko o", kp=P)
    wor = wot[:].rearrange("(ko kp) o -> kp ko o", kp=P)
    outr = out[:].rearrange("(o p) l -> p o l", p=P)    # (128, 8, 512)

    with tile.TileContext(nc) as tc:
        with (
            tc.tile_pool(name="consts", bufs=1) as consts,
            tc.tile_pool(name="resident", bufs=1) as res,
            tc.tile_pool(name="wstream", bufs=3) as wpool,
            tc.tile_pool(name="exp", bufs=8) as epool,
            tc.tile_pool(name="norm", bufs=2) as npool,
            tc.tile_pool(name="outp", bufs=3) as opool,
            tc.tile_pool(name="ps_proj", bufs=2, space="PSUM") as ps_proj,
            tc.tile_pool(name="ps_sc", bufs=2, space="PSUM") as ps_sc,
            tc.tile_pool(name="ps_c", bufs=2, space="PSUM") as ps_c,
        ):
            # ---- small inputs first: xq (sync/HWDGE queue, fast) unblocks
            # the Q projection; bulk loads go on the gpsimd queue. ----
            xq_sb = res.tile([P, DC, LQ], BF16)
            xq_dma = nc.sync.dma_start(out=xq_sb[:], in_=xqr)
            # selector for per-pair denominator broadcast: selp[j, p] = 1 iff p//64 == j
            selp_sb = consts.tile([2, P], F32)
            nc.sync.dma_start(out=selp_sb[:], in_=selp[:])


            k_sb = res.tile([P, DC, L], BF16)     # K   (D x L)
            q_sb = res.tile([P, DC, LQ], BF16)    # Q   (D x LQ)
            c_sb = res.tile([P, DC, LQ], F32)     # C   (D x LQ) unnormalized
            cn_sb = res.tile([P, DC, LQ], BF16)   # C   normalized (matmul input)
            vt_sb = res.tile([P, LT, H * HV], BF16)  # V^T tiles + ones cols

            vt4 = vt_sb[:].rearrange("p l (h e) -> p l h e", e=HV)
            nc.vector.memset(vt4[:, :, :, DH : DH + 1], 1.0)

            # ---- Phase 1: Q projection (small, unblocks attention early) ----
            wq_dmas = []
            for mt in range(DC):
                wt = wpool.tile([P, DC, P], BF16, tag="w")
                wq_dmas.append(
                    nc.sync.dma_start(out=wt[:], in_=wqr[:, :, mt * P : (mt + 1) * P])
                )
                ps = ps_proj.tile([P, LQ], F32, tag="proj")
                for kt in range(DC):
                    nc.tensor.matmul(
                        ps[:],
                        lhsT=wt[:, kt, :],
                        rhs=xq_sb[:, kt, :],
                        start=(kt == 0),
                        stop=(kt == DC - 1),
                    )
                nc.vector.tensor_copy(out=q_sb[:, mt, :], in_=ps[:])

            # ---- bulk loads: every chunk gated behind the startup-critical
            # xq; xb/wvt interleaved pairwise so the V^T projection can start
            # consuming chunk k as soon as pair k has landed ----
            xb = res.tile([P, DC, L], BF16)       # x[b]  (channels-first)
            wvt_sb = res.tile([P, DC, D], BF16)   # Wv.T resident
            for kt in range(DC):
                dma = nc.gpsimd.dma_start(out=xb[:, kt, :], in_=xr[:, kt, :])
                add_dep_helper(dma.ins, xq_dma.ins, reason="startup order")
                dma = nc.gpsimd.dma_start(out=wvt_sb[:, kt, :], in_=wvr[:, kt, :])
                add_dep_helper(dma.ins, xq_dma.ins, reason="startup order")

            # ---- Phase 2: V^T projection ----
            for lt in range(LT):
                for oc in range(2):
                    ps = ps_proj.tile([P, LQ], F32, tag="proj")
                    for kt in range(DC):
                        nc.tensor.matmul(
                            ps[:],
                            lhsT=xb[:, kt, lt * P : (lt + 1) * P],
                            rhs=wvt_sb[:, kt, oc * 512 : (oc + 1) * 512],
                            start=(kt == 0),
                            stop=(kt == DC - 1),
                        )
                    dest = vt4[:, lt, oc * 8 : (oc + 1) * 8, 0:DH]
                    nc.vector.tensor_copy(
                        out=dest, in_=ps[:].rearrange("p (h e) -> p h e", e=DH)
                    )

            # ---- Phase 3: per mt: K projection, then attention for its two
            # heads.  Interleaving keeps the scalar engine (exp) fed while the
            # tensor engine grinds projections, and the two heads' score
            # matmuls (K=64 at partition bases 0 and 64) run concurrently on
            # disjoint PE row groups. ----
            for mt in range(DC):
                wt = wpool.tile([P, DC, P], BF16, tag="w")
                nc.sync.dma_start(out=wt[:], in_=wkr[:, :, mt * P : (mt + 1) * P])
                for ncol in range(L // LQ):
                    ps = ps_proj.tile([P, LQ], F32, tag="proj")
                    for kt in range(DC):
                        nc.tensor.matmul(
                            ps[:],
                            lhsT=wt[:, kt, :],
                            rhs=xb[:, kt, ncol * LQ : (ncol + 1) * LQ],
                            start=(kt == 0),
                            stop=(kt == DC - 1),
                        )
                    nc.vector.tensor_copy(
                        out=k_sb[:, mt, ncol * LQ : (ncol + 1) * LQ], in_=ps[:]
                    )

                # Attention for heads (2mt, 2mt+1).  Both heads' scores for
                # one kt share a single (128, 1024) psum tile: one exp covers
                # both, the pool double-buffers across kt, and the two score
                # matmuls (row groups 0-1 vs 2-3 via partition bases 0/64)
                # issue back-to-back so they run concurrently in the array.
                ha, hb = 2 * mt, 2 * mt + 1
                c_ps_a = ps_c.tile([HV, LQ], F32, tag="c")
                c_ps_b = ps_c.tile([HV, LQ], F32, tag="c")
                for kt in range(LT):
                    s_ab = ps_sc.tile([P, 2 * LQ], F32, tag="sc")
                    nc.tensor.matmul(
                        s_ab[:, 0:LQ],
                        lhsT=k_sb[0:DH, mt, kt * P : (kt + 1) * P],
                        rhs=q_sb[0:DH, mt, :],
                        start=True,
                        stop=True,
                    )
                    nc.tensor.matmul(
                        s_ab[:, LQ : 2 * LQ],
                        lhsT=k_sb[DH:P, mt, kt * P : (kt + 1) * P],
                        rhs=q_sb[DH:P, mt, :],
                        start=True,
                        stop=True,
                    )
                    e_ab = epool.tile([P, 2 * LQ], BF16, tag="e")
                    nc.scalar.activation(e_ab[:], s_ab[:], AF.Exp, scale=float(SCALE))
                    nc.tensor.matmul(
                        c_ps_a[:],
                        lhsT=vt_sb[:, kt, ha * HV : (ha + 1) * HV],
                        rhs=e_ab[:, 0:LQ],
                        start=(kt == 0),
                        stop=(kt == LT - 1),
                    )
                    nc.tensor.matmul(
                        c_ps_b[:],
                        lhsT=vt_sb[:, kt, hb * HV : (hb + 1) * HV],
                        rhs=e_ab[:, LQ : 2 * LQ],
                        start=(kt == 0),
                        stop=(kt == LT - 1),
                    )
                # ---- per-pair normalization: stage both denom rows into a
                # (2, LQ) tile (via DMA: engine APs cannot write partition 1),
                # one reciprocal, one K=2 broadcast matmul (psum slot from the
                # just-released ps_c pool, so projection psum is not starved),
                # one multiply. ----
                den_pair = npool.tile([2, LQ], F32, tag="den")
                for h, c_ps in ((ha, c_ps_a), (hb, c_ps_b)):
                    po = (h % 2) * DH
                    nc.vector.tensor_copy(
                        out=c_sb[po : po + DH, mt, :], in_=c_ps[0:DH, :]
                    )
                    stage = npool.tile([1, LQ], F32, tag="stage")
                    nc.vector.tensor_copy(out=stage[:], in_=c_ps[DH : DH + 1, :])
                    nc.sync.dma_start(
                        out=den_pair[h % 2 : h % 2 + 1, :], in_=stage[:]
                    )
                recip = npool.tile([2, LQ], F32, tag="recip")
                nc.vector.reciprocal(recip[:], den_pair[:])
                bc_ps = ps_c.tile([P, LQ], F32, tag="c")
                nc.tensor.matmul(
                    bc_ps[:], lhsT=selp_sb[:], rhs=recip[:], start=True, stop=True
                )
                nc.vector.tensor_mul(
                    out=cn_sb[:, mt, :], in0=c_sb[:, mt, :], in1=bc_ps[:]
                )

            # ---- Phase 5: output projection ----
            for mt in range(DC):
                wt = wpool.tile([P, DC, P], BF16, tag="w")
                nc.sync.dma_start(out=wt[:], in_=wor[:, :, mt * P : (mt + 1) * P])
                ps = ps_proj.tile([P, LQ], F32, tag="proj")
                for kt in range(DC):
                    nc.tensor.matmul(
                        ps[:],
                        lhsT=wt[:, kt, :],
                        rhs=cn_sb[:, kt, :],
                        start=(kt == 0),
                        stop=(kt == DC - 1),
                    )
                o_sb = opool.tile([P, LQ], F32, tag="o")
                nc.vector.tensor_copy(out=o_sb[:], in_=ps[:])
                nc.sync.dma_start(out=outr[:, mt, :], in_=o_sb[:])

    if not nc.is_finalized():
        nc.finalize()
    return nc


_NC_CACHE = {}


def _get_nc():
    if "nc" not in _NC_CACHE:
        _NC_CACHE["nc"] = build()
    return _NC_CACHE["nc"]


def _run(x, Wq, Wk, Wv, Wo, trace=False):
    """x: (B, D, L) f32; W*: (D, D) f32. Returns (out, BassKernelResults)."""
    nc = _get_nc()
    bf = ml_dtypes.bfloat16
    xb = np.ascontiguousarray(x).astype(bf)                 # (B, D, L)
    wqt = np.ascontiguousarray(np.asarray(Wq, np.float32).T).astype(bf)
    wkt = np.ascontiguousarray(np.asarray(Wk, np.float32).T).astype(bf)
    wvt = np.ascontiguousarray(np.asarray(Wv, np.float32).T).astype(bf)
    wot = np.ascontiguousarray(np.asarray(Wo, np.float32).T).astype(bf)

    selp = np.zeros((2, P), np.float32)
    selp[0, 0:DH] = 1.0
    selp[1, DH:P] = 1.0

    in_maps = []
    for c in range(8):
        b = c // 4
        q0 = (c % 4) * LQ
        in_maps.append(
            {
                "x": xb[b],
                "xq": np.ascontiguousarray(xb[b][:, q0 : q0 + LQ]),
                "wqt": wqt,
                "wkt": wkt,
                "wvt": wvt,
                "wot": wot,
                "selp": selp,
            }
        )
    res = run_bass_kernel_spmd(nc, in_maps, core_ids=list(range(8)), trace=trace)
    out = np.empty((B, D, L), np.float32)
    for c in range(8):
        b = c // 4
        q0 = (c % 4) * LQ
        out[b][:, q0 : q0 + LQ] = res.results[c]["out"]
    return out, res


def kernel(x, mask, Wq, Wk, Wv, Wo):
    # mask is all-ones by construction (fill: ones) -- softmax over all keys.
    out, _ = _run(x, Wq, Wk, Wv, Wo, trace=False)
    return out



# revision 3
# speedup vs baseline: 1.3002x; 1.3002x over previous
"""Multi-head attention (B=2, D=1024, L=2048, H=16) on 8 TRN2 NeuronCores.

Sharding (per spec hint): tensor-parallel over heads x data-parallel over
batch.  Core c handles batch c//4 and head group g=c%4 (4 heads: 2 "pairs"
of 2 heads stacked on 64-partition halves).  Each core:
  - projects Q/K for its 2 head-pairs (dh on partitions, pair-stacked),
  - projects V directly in transposed layout VT (Lk x dh) with a ones
    column per head so A@V also emits the softmax denominator,
  - computes scores transposed ST[k,q] per pair with the two heads'
    K=64 matmuls issued back-to-back at partition bases 0/64 (they run
    concurrently on disjoint PE row groups -> ~2x),
  - exp on the scalar engine (the only exp-capable engine; attention is
    ACT-paced, so all other PE work is scheduled to hide under it),
  - A@V accumulated over all 16 k-tiles in PSUM,
  - normalization: denominator rows -> reciprocal -> K=2 selector matmul
    broadcast -> one multiply; emission deferred by one attention unit so
    PE never blocks on the DVE reciprocal,
  - row-parallel output projection (contraction = its 256 head dims) ->
    bf16 PARTIAL output; host sums the 4 partials per batch in f32.
    Out-proj for query-block qb is emitted after qb+1's attention so it
    fills the ACT-bound pipeline instead of stalling it.

All matmuls bf16 (f32 PSUM accumulate); softmax stats f32.
"""

import sys
import types

import numpy as np
import ml_dtypes


def _install_axon_hooks_shim():
    """antenv.axon_hooks is absent in this image; concourse imports it when
    tracing is requested (e.g. via the BASS_TRACE env var)."""
    try:
        import antenv.axon_hooks  # noqa: F401
        return
    except ImportError:
        pass
    try:
        import antenv
    except ImportError:
        return
    mod = types.ModuleType("antenv.axon_hooks")
    mod._hook = None
    mod.set_axon_ntff_profile_hook = lambda h: setattr(mod, "_hook", h)
    mod.get_axon_ntff_profile_hook = lambda: mod._hook
    sys.modules["antenv.axon_hooks"] = mod
    antenv.axon_hooks = mod
    try:
        from trn_agent_boot.trn_boot import _ntff_profile_via_ctypes

        h = _ntff_profile_via_ctypes("/opt/axon/libaxon_pjrt.so")
        if h is not None:
            mod._hook = h
    except Exception:
        pass


_install_axon_hooks_shim()

import concourse.bass as bass  # noqa: E402
import concourse.mybir as mybir  # noqa: E402
import concourse.tile as tile  # noqa: E402
from concourse import bacc  # noqa: E402
from concourse.bass_utils import run_bass_kernel_spmd  # noqa: E402

BF16 = mybir.dt.bfloat16
F32 = mybir.dt.float32
AF = mybir.ActivationFunctionType

B, D, L, H = 2, 1024, 2048, 16
DH = D // H            # 64
P = 128
SCALE = 1.0 / np.sqrt(np.float32(DH))

DC = D // P            # 8 contraction chunks over D
LT = L // P            # 16 Lk tiles
HV = DH + 1            # V^T per-head width incl. ones column
NPAIR = 2              # head pairs per core (4 heads)
NQB = 4                # query blocks of 512
QB = L // NQB          # 512


def build():
    nc = bacc.Bacc(None, target_bir_lowering=False, debug=False)

    x = nc.dram_tensor("x", [D, L], BF16, kind="ExternalInput")
    wq = nc.dram_tensor("wq", [D, NPAIR * P], BF16, kind="ExternalInput")
    wk = nc.dram_tensor("wk", [D, NPAIR * P], BF16, kind="ExternalInput")
    wv = nc.dram_tensor("wv", [D, NPAIR * P], BF16, kind="ExternalInput")
    wo = nc.dram_tensor("wo", [NPAIR * P, D], BF16, kind="ExternalInput")
    selp = nc.dram_tensor("selp", [2, P], BF16, kind="ExternalInput")
    out = nc.dram_tensor("out", [D, L], BF16, kind="ExternalOutput")

    xr = x[:].rearrange("(o p) l -> p o l", p=P)          # (128, 8, 2048)
    wqr = wq[:].rearrange("(o p) m -> p o m", p=P)        # (128, 8, 256)
    wkr = wk[:].rearrange("(o p) m -> p o m", p=P)
    wvr = wv[:].rearrange("(o p) m -> p o m", p=P)
    wor = wo[:].rearrange("(j p) o -> p j o", p=P)        # (128, 2, 1024)
    outr = out[:].rearrange("(o p) l -> p o l", p=P)      # (128, 8, 2048)

    with tile.TileContext(nc) as tc:
        with (
            tc.tile_pool(name="consts", bufs=1) as consts,
            tc.tile_pool(name="res", bufs=1) as res,
            tc.tile_pool(name="exp", bufs=3) as epool,
            tc.tile_pool(name="norm", bufs=2) as npool,
            tc.tile_pool(name="outp", bufs=3) as opool,
            tc.tile_pool(name="ps_s", bufs=2, space="PSUM") as ps_s,
            tc.tile_pool(name="ps_c", bufs=3, space="PSUM") as ps_c,
            tc.tile_pool(name="ps_o", bufs=1, space="PSUM") as ps_o,
        ):
            selp_sb = consts.tile([2, P], BF16)
            nc.sync.dma_start(out=selp_sb[:], in_=selp[:])
            wq_sb = res.tile([P, DC, NPAIR * P], BF16)
            nc.sync.dma_start(out=wq_sb[:], in_=wqr)
            wk_sb = res.tile([P, DC, NPAIR * P], BF16)
            nc.sync.dma_start(out=wk_sb[:], in_=wkr)

            xb = res.tile([P, DC, L], BF16)
            for kt in range(DC):
                nc.gpsimd.dma_start(out=xb[:, kt, :], in_=xr[:, kt, :])
            wv_sb = res.tile([P, DC, NPAIR * P], BF16)
            nc.gpsimd.dma_start(out=wv_sb[:], in_=wvr)
            wo_sb = res.tile([P, NPAIR, D], BF16)
            nc.gpsimd.dma_start(out=wo_sb[:], in_=wor)

            q_sb = res.tile([P, NPAIR, L], BF16)
            k_sb = res.tile([P, NPAIR, L], BF16)
            vt_sb = res.tile([P, LT, 2 * NPAIR * HV], BF16)
            vt4 = vt_sb[:].rearrange("p l (h e) -> p l h e", e=HV)
            nc.vector.memset(vt4[:, :, :, DH : DH + 1], 1.0)
            c_sb = res.tile([P, NPAIR, L], F32)     # unnormalized C
            cn_sb = res.tile([P, NPAIR, L], BF16)   # normalized C

            # ---- Q/K projections: kt-outer (ldweights reuse), psum holds a
            # full (128, 2048) pair row via 2x (128,1024) ps_s tiles ----
            def proj_pair(w_sb, dst, j):
                psA = ps_s.tile([P, 2 * QB], F32, tag="s")
                psB = ps_s.tile([P, 2 * QB], F32, tag="s")
                for kt in range(DC):
                    lhsT = w_sb[:, kt, j * P : (j + 1) * P]
                    for half, ps in ((0, psA), (1, psB)):
                        for cb in range(2):
                            n0 = cb * QB
                            nc.tensor.matmul(
                                ps[:, n0 : n0 + QB],
                                lhsT=lhsT,
                                rhs=xb[:, kt, half * 1024 + n0 : half * 1024 + n0 + QB],
                                start=(kt == 0),
                                stop=(kt == DC - 1),
                            )
                nc.vector.tensor_copy(out=dst[:, j, 0:1024], in_=psA[:])
                nc.vector.tensor_copy(out=dst[:, j, 1024:2048], in_=psB[:])

            for j in range(NPAIR):
                proj_pair(wq_sb, q_sb, j)
            for j in range(NPAIR):
                proj_pair(wk_sb, k_sb, j)

            # ---- V projection straight into VT layout ----
            for lt in range(LT):
                psv = ps_c.tile([P, 2 * NPAIR * DH], F32, tag="c")
                for kt in range(DC):
                    nc.tensor.matmul(
                        psv[:],
                        lhsT=xb[:, kt, lt * P : (lt + 1) * P],
                        rhs=wv_sb[:, kt, :],
                        start=(kt == 0),
                        stop=(kt == DC - 1),
                    )
                nc.vector.tensor_copy(
                    out=vt4[:, lt, :, 0:DH],
                    in_=psv[:].rearrange("p (h e) -> p h e", e=DH),
                )

            # ---- attention units: (qb, pair), with deferred norm_b and
            # out-proj emission so the PE queue never waits on DVE ----
            state = {}

            def emit_attention(qb, j):
                c_a = ps_c.tile([HV, QB], F32, tag="c")
                c_b = ps_c.tile([HV, QB], F32, tag="c")
                q0 = qb * QB
                for t in range(LT):
                    s = ps_s.tile([P, 2 * QB], F32, tag="s")
                    nc.tensor.matmul(
                        s[:, 0:QB],
                        lhsT=k_sb[0:DH, j, t * P : (t + 1) * P],
                        rhs=q_sb[0:DH, j, q0 : q0 + QB],
                        start=True,
                        stop=True,
                    )
                    nc.tensor.matmul(
                        s[:, QB : 2 * QB],
                        lhsT=k_sb[DH:P, j, t * P : (t + 1) * P],
                        rhs=q_sb[DH:P, j, q0 : q0 + QB],
                        start=True,
                        stop=True,
                    )
                    e = epool.tile([P, 2 * QB], BF16, tag="e")
                    nc.scalar.activation(e[:], s[:], AF.Exp, scale=float(SCALE))
                    nc.tensor.matmul(
                        c_a[:],
                        lhsT=vt4[:, t, 2 * j, :],
                        rhs=e[:, 0:QB],
                        start=(t == 0),
                        stop=(t == LT - 1),
                    )
                    nc.tensor.matmul(
                        c_b[:],
                        lhsT=vt4[:, t, 2 * j + 1, :],
                        rhs=e[:, QB : 2 * QB],
                        start=(t == 0),
                        stop=(t == LT - 1),
                    )
                state[(qb, j)] = (c_a, c_b)

            def emit_norm_a(qb, j):
                # DVE-only: drain C, stage denominators, reciprocal.
                c_a, c_b = state[(qb, j)]
                q0 = qb * QB
                nc.vector.tensor_copy(
                    out=c_sb[0:DH, j, q0 : q0 + QB], in_=c_a[0:DH, :]
                )
                nc.vector.tensor_copy(
                    out=c_sb[DH:P, j, q0 : q0 + QB], in_=c_b[0:DH, :]
                )
                den = npool.tile([2, QB], F32, tag="den")
                nc.vector.tensor_copy(out=den[0:1, :], in_=c_a[DH : DH + 1, :])
                stage = npool.tile([1, QB], F32, tag="stg")
                nc.vector.tensor_copy(out=stage[:], in_=c_b[DH : DH + 1, :])
                nc.sync.dma_start(out=den[1:2, :], in_=stage[:])
                recip = npool.tile([2, QB], BF16, tag="rcp")
                with nc.allow_low_precision(reason="bf16 1/den ok for 2e-2 tol"):
                    nc.vector.reciprocal(recip[:], den[:])
                state[(qb, j, "r")] = recip

            def emit_norm_b(qb, j):
                # K=2 broadcast matmul + one multiply -> normalized C.
                recip = state.pop((qb, j, "r"))
                c_a, c_b = state.pop((qb, j))
                q0 = qb * QB
                bc = ps_c.tile([P, QB], F32, tag="c")
                nc.tensor.matmul(
                    bc[:], lhsT=selp_sb[:], rhs=recip[:], start=True, stop=True
                )
                nc.vector.tensor_mul(
                    out=cn_sb[:, j, q0 : q0 + QB],
                    in0=c_sb[:, j, q0 : q0 + QB],
                    in1=bc[:],
                )

            def emit_outproj(qb, tail=False):
                q0 = qb * QB
                for mt in range(DC):
                    if tail and mt % 2 == 1:
                        po = ps_c.tile([P, QB], F32, tag="c")
                    else:
                        po = ps_o.tile([P, QB], F32, tag="o")
                    for j in range(NPAIR):
                        nc.tensor.matmul(
                            po[:],
                            lhsT=wo_sb[:, j, mt * P : (mt + 1) * P],
                            rhs=cn_sb[:, j, q0 : q0 + QB],
                            start=(j == 0),
                            stop=(j == NPAIR - 1),
                        )
                    o_t = opool.tile([P, QB], BF16, tag="ot")
                    nc.vector.tensor_copy(out=o_t[:], in_=po[:])
                    nc.gpsimd.dma_start(out=outr[:, mt, q0 : q0 + QB], in_=o_t[:])

            units = [(qb, j) for qb in range(NQB) for j in range(NPAIR)]
            for idx, (qb, j) in enumerate(units):
                emit_attention(qb, j)
                emit_norm_a(qb, j)
                if idx >= 1:
                    emit_norm_b(*units[idx - 1])
                if idx >= 3 and idx % 2 == 1:
                    emit_outproj(qb - 1)
            emit_norm_b(*units[-1])
            emit_outproj(NQB - 1, tail=True)

    if not nc.is_finalized():
        nc.finalize()
    return nc


_NC_CACHE = {}


def _get_nc():
    if "nc" not in _NC_CACHE:
        _NC_CACHE["nc"] = build()
    return _NC_CACHE["nc"]


def _run(x, Wq, Wk, Wv, Wo, trace=False):
    """x: (B, D, L) f32; W*: (D, D) f32. Returns (out, BassKernelResults)."""
    nc = _get_nc()
    bf = ml_dtypes.bfloat16
    xb = np.ascontiguousarray(x).astype(bf)                 # (B, D, L)
    wqt = np.ascontiguousarray(np.asarray(Wq, np.float32).T).astype(bf)
    wkt = np.ascontiguousarray(np.asarray(Wk, np.float32).T).astype(bf)
    wvt = np.ascontiguousarray(np.asarray(Wv, np.float32).T).astype(bf)
    wot = np.ascontiguousarray(np.asarray(Wo, np.float32).T).astype(bf)

    selp = np.zeros((2, P), np.float32)
    selp[0, 0:DH] = 1.0
    selp[1, DH:P] = 1.0
    selp = selp.astype(bf)

    in_maps = []
    for c in range(8):
        b = c // 4
        g = c % 4
        r0 = g * NPAIR * P
        in_maps.append(
            {
                "x": xb[b],
                "wq": np.ascontiguousarray(wqt[:, r0 : r0 + NPAIR * P]),
                "wk": np.ascontiguousarray(wkt[:, r0 : r0 + NPAIR * P]),
                "wv": np.ascontiguousarray(wvt[:, r0 : r0 + NPAIR * P]),
                "wo": np.ascontiguousarray(wot[r0 : r0 + NPAIR * P, :]),
                "selp": selp,
            }
        )
    res = run_bass_kernel_spmd(nc, in_maps, core_ids=list(range(8)), trace=trace)
    out = np.zeros((B, D, L), np.float32)
    for c in range(8):
        b = c // 4
        out[b] += res.results[c]["out"].astype(np.float32)
    return out, res


def kernel(x, mask, Wq, Wk, Wv, Wo):
    # mask is all-ones by construction (fill: ones) -- softmax over all keys.
    out, _ = _run(x, Wq, Wk, Wv, Wo, trace=False)
    return out


# revision 6
# speedup vs baseline: 1.3605x; 1.0464x over previous
"""Multi-head attention (B=2, D=1024, L=2048, H=16) on 8 TRN2 NeuronCores.

Sharding (per spec hint): tensor-parallel over heads x data-parallel over
batch.  Core c handles batch c//4 and head group g=c%4 (4 heads as 2
"pairs" of 2 heads stacked on 64-partition halves).  Host sums the 4
bf16 partial outputs per batch (row-parallel W_O) in f32 -- no on-device
collective.

Per core:
  - Q/K projected into pair layout (head dims on partitions); V projected
    directly into transposed layout VT (Lk x dh) with a ones column per
    head so A@V also emits the softmax denominator.
  - Scores ST[k,q]: the two heads' K=64 matmuls are issued back-to-back
    at partition bases 0/64; they run concurrently on disjoint PE row
    groups (~2x, verified in trace).
  - exp runs on the scalar/ACT engine (the only exp engine): 128 x
    (128,1024) activations ~= 142us is the kernel's pacing floor.  All
    other PE work (pair-1 Q/K projections, output projection) is emitted
    as fillers INSIDE attention units so it executes in the PE's
    ACT-bound gaps instead of serializing.
  - Normalization per (pair, qb): denominator rows -> DVE reciprocal ->
    K=2 selector matmul broadcast -> one multiply.  The matmul+multiply
    (norm_b) is emitted one unit later so the PE queue never waits on
    the (slow, ~3.3us) DVE reciprocal.

All matmuls bf16 (f32 PSUM accumulate); softmax stats f32.
"""

import sys
import types

import numpy as np
import ml_dtypes


def _install_axon_hooks_shim():
    try:
        import antenv.axon_hooks  # noqa: F401
        return
    except ImportError:
        pass
    try:
        import antenv
    except ImportError:
        return
    mod = types.ModuleType("antenv.axon_hooks")
    mod._hook = None
    mod.set_axon_ntff_profile_hook = lambda h: setattr(mod, "_hook", h)
    mod.get_axon_ntff_profile_hook = lambda: mod._hook
    sys.modules["antenv.axon_hooks"] = mod
    antenv.axon_hooks = mod
    try:
        from trn_agent_boot.trn_boot import _ntff_profile_via_ctypes

        h = _ntff_profile_via_ctypes("/opt/axon/libaxon_pjrt.so")
        if h is not None:
            mod._hook = h
    except Exception:
        pass


_install_axon_hooks_shim()

import concourse.bass as bass  # noqa: E402
import concourse.mybir as mybir  # noqa: E402
import concourse.tile as tile  # noqa: E402
from concourse import bacc  # noqa: E402
from concourse.bass_utils import run_bass_kernel_spmd  # noqa: E402

BF16 = mybir.dt.bfloat16
F32 = mybir.dt.float32
AF = mybir.ActivationFunctionType

B, D, L, H = 2, 1024, 2048, 16
DH = D // H            # 64
P = 128
SCALE = 1.0 / np.sqrt(np.float32(DH))

DC = D // P            # 8 contraction chunks over D
LT = L // P            # 16 Lk tiles
HV = DH + 1            # V^T per-head width incl. ones column
NPAIR = 2              # head pairs per core (4 heads)
NQB = 4                # query blocks of 512
QB = L // NQB          # 512


def build():
    nc = bacc.Bacc(None, target_bir_lowering=False, debug=False)

    x = nc.dram_tensor("x", [D, L], BF16, kind="ExternalInput")
    wq = nc.dram_tensor("wq", [D, NPAIR * P], BF16, kind="ExternalInput")
    wk = nc.dram_tensor("wk", [D, NPAIR * P], BF16, kind="ExternalInput")
    wv = nc.dram_tensor("wv", [D, NPAIR * P], BF16, kind="ExternalInput")
    wo = nc.dram_tensor("wo", [NPAIR * P, D], BF16, kind="ExternalInput")
    selp = nc.dram_tensor("selp", [2, P], BF16, kind="ExternalInput")
    out = nc.dram_tensor("out", [D, L], BF16, kind="ExternalOutput")

    xr = x[:].rearrange("(o p) l -> p o l", p=P)          # (128, 8, 2048)
    wqr = wq[:].rearrange("(o p) m -> p o m", p=P)        # (128, 8, 256)
    wkr = wk[:].rearrange("(o p) m -> p o m", p=P)
    wvr = wv[:].rearrange("(o p) m -> p o m", p=P)
    wor = wo[:].rearrange("(j p) o -> p j o", p=P)        # (128, 2, 1024)
    outr = out[:].rearrange("(o p) l -> p o l", p=P)      # (128, 8, 2048)

    with tile.TileContext(nc) as tc:
        with (
            tc.tile_pool(name="consts", bufs=1) as consts,
            tc.tile_pool(name="res", bufs=1) as res,
            tc.tile_pool(name="exp", bufs=3) as epool,
            tc.tile_pool(name="norm", bufs=2) as npool,
            tc.tile_pool(name="outp", bufs=3) as opool,
            tc.tile_pool(name="ps_s", bufs=2, space="PSUM") as ps_s,
            tc.tile_pool(name="ps_c", bufs=3, space="PSUM") as ps_c,
            tc.tile_pool(name="ps_f", bufs=1, space="PSUM") as ps_f,
        ):
            selp_sb = consts.tile([2, P], BF16)
            nc.sync.dma_start(out=selp_sb[:], in_=selp[:])
            wq_sb = res.tile([P, DC, NPAIR * P], BF16)
            nc.sync.dma_start(out=wq_sb[:], in_=wqr)
            wk_sb = res.tile([P, DC, NPAIR * P], BF16)
            nc.sync.dma_start(out=wk_sb[:], in_=wkr)

            # x split across the two DMA paths: even chunks on the gpsimd
            # (software-DGE) queue start immediately; odd chunks follow the
            # small weight loads on the sync (HWDGE) queue.
            xb = res.tile([P, DC, L], BF16)
            for kt in range(DC):
                eng = nc.gpsimd if kt % 2 == 0 else nc.sync
                eng.dma_start(out=xb[:, kt, :], in_=xr[:, kt, :])
            wv_sb = res.tile([P, DC, NPAIR * P], BF16)
            nc.gpsimd.dma_start(out=wv_sb[:], in_=wvr)
            wo_sb = res.tile([P, NPAIR, D], BF16)
            nc.gpsimd.dma_start(out=wo_sb[:], in_=wor)

            q_sb = res.tile([P, NPAIR, L], BF16)
            k_sb = res.tile([P, NPAIR, L], BF16)
            vt_sb = res.tile([P, LT, 2 * NPAIR * HV], BF16)
            vt4 = vt_sb[:].rearrange("p l (h e) -> p l h e", e=HV)
            nc.vector.memset(vt4[:, :, :, DH : DH + 1], 1.0)
            c_sb = res.tile([P, NPAIR, L], F32)     # unnormalized C
            cn_sb = res.tile([P, NPAIR, L], BF16)   # normalized C

            # ---- upfront: pair-0 Q/K projections (kt-outer, ldweights
            # reuse across the 4 q-columns), full V projection ----
            def proj_pair(w_sb, dst, j):
                psA = ps_s.tile([P, 2 * QB], F32, tag="s")
                psB = ps_s.tile([P, 2 * QB], F32, tag="s")
                for kt in range(DC):
                    lhsT = w_sb[:, kt, j * P : (j + 1) * P]
                    for half, ps in ((0, psA), (1, psB)):
                        for cb in range(2):
                            n0 = cb * QB
                            nc.tensor.matmul(
                                ps[:, n0 : n0 + QB],
                                lhsT=lhsT,
                                rhs=xb[:, kt, half * 1024 + n0 : half * 1024 + n0 + QB],
                                start=(kt == 0),
                                stop=(kt == DC - 1),
                            )
                nc.vector.tensor_copy(out=dst[:, j, 0:1024], in_=psA[:])
                nc.vector.tensor_copy(out=dst[:, j, 1024:2048], in_=psB[:])

            proj_pair(wq_sb, q_sb, 0)
            proj_pair(wk_sb, k_sb, 0)

            for lt in range(LT):
                psv = ps_c.tile([P, 2 * NPAIR * DH], F32, tag="c")
                for kt in range(DC):
                    nc.tensor.matmul(
                        psv[:],
                        lhsT=xb[:, kt, lt * P : (lt + 1) * P],
                        rhs=wv_sb[:, kt, :],
                        start=(kt == 0),
                        stop=(kt == DC - 1),
                    )
                nc.vector.tensor_copy(
                    out=vt4[:, lt, :, 0:DH],
                    in_=psv[:].rearrange("p (h e) -> p h e", e=DH),
                )

            # ---- filler generators (run inside attention units) ----
            def mk_proj_col(w_sb, dst, col):
                # one 512-wide column of the pair-1 Q or K projection
                def f():
                    pc = ps_f.tile([P, QB], F32, tag="f")
                    for kt in range(DC):
                        nc.tensor.matmul(
                            pc[:],
                            lhsT=w_sb[:, kt, P : 2 * P],
                            rhs=xb[:, kt, col * QB : (col + 1) * QB],
                            start=(kt == 0),
                            stop=(kt == DC - 1),
                        )
                    nc.vector.tensor_copy(
                        out=dst[:, 1, col * QB : (col + 1) * QB], in_=pc[:]
                    )
                return f

            def mk_outproj_mt(qb, mt, pool=None):
                # one 128-row block of the output projection for query block qb
                def f():
                    po = (pool or ps_f).tile([P, QB], F32, tag="f" if pool is None else "c")
                    q0 = qb * QB
                    for j in range(NPAIR):
                        nc.tensor.matmul(
                            po[:],
                            lhsT=wo_sb[:, j, mt * P : (mt + 1) * P],
                            rhs=cn_sb[:, j, q0 : q0 + QB],
                            start=(j == 0),
                            stop=(j == NPAIR - 1),
                        )
                    o_t = opool.tile([P, QB], BF16, tag="ot")
                    nc.vector.tensor_copy(out=o_t[:], in_=po[:])
                    nc.sync.dma_start(out=outr[:, mt, q0 : q0 + QB], in_=o_t[:])
                return f

            # ---- attention unit with interleaved fillers ----
            state = {}

            def emit_attention(qb, j, fillers=(), stride=4):
                c_a = ps_c.tile([HV, QB], F32, tag="c")
                c_b = ps_c.tile([HV, QB], F32, tag="c")
                fl = list(fillers)
                q0 = qb * QB
                for t in range(LT):
                    s = ps_s.tile([P, 2 * QB], F32, tag="s")
                    nc.tensor.matmul(
                        s[:, 0:QB],
                        lhsT=k_sb[0:DH, j, t * P : (t + 1) * P],
                        rhs=q_sb[0:DH, j, q0 : q0 + QB],
                        start=True,
                        stop=True,
                    )
                    nc.tensor.matmul(
                        s[:, QB : 2 * QB],
                        lhsT=k_sb[DH:P, j, t * P : (t + 1) * P],
                        rhs=q_sb[DH:P, j, q0 : q0 + QB],
                        start=True,
                        stop=True,
                    )
                    e = epool.tile([P, 2 * QB], BF16, tag="e")
                    nc.scalar.activation(e[:], s[:], AF.Exp, scale=float(SCALE))
                    nc.tensor.matmul(
                        c_a[:],
                        lhsT=vt4[:, t, 2 * j, :],
                        rhs=e[:, 0:QB],
                        start=(t == 0),
                        stop=(t == LT - 1),
                    )
                    nc.tensor.matmul(
                        c_b[:],
                        lhsT=vt4[:, t, 2 * j + 1, :],
                        rhs=e[:, QB : 2 * QB],
                        start=(t == 0),
                        stop=(t == LT - 1),
                    )
                    if fl and t % stride == stride - 1:
                        fl.pop(0)()
                for f in fl:
                    f()
                state[(qb, j)] = (c_a, c_b)

            def emit_norm_a(qb, j):
                # DVE-only: drain C (frees the c ring first), stage
                # denominators, reciprocal.
                c_a, c_b = state[(qb, j)]
                q0 = qb * QB
                nc.vector.tensor_copy(
                    out=c_sb[0:DH, j, q0 : q0 + QB], in_=c_a[0:DH, :]
                )
                nc.vector.tensor_copy(
                    out=c_sb[DH:P, j, q0 : q0 + QB], in_=c_b[0:DH, :]
                )
                den = npool.tile([2, QB], F32, tag="den")
                nc.vector.tensor_copy(out=den[0:1, :], in_=c_a[DH : DH + 1, :])
                stage = npool.tile([1, QB], F32, tag="stg")
                nc.vector.tensor_copy(out=stage[:], in_=c_b[DH : DH + 1, :])
                nc.sync.dma_start(out=den[1:2, :], in_=stage[:])
                recip = npool.tile([2, QB], BF16, tag="rcp")
                with nc.allow_low_precision(reason="bf16 1/den ok for 2e-2 tol"):
                    nc.vector.reciprocal(recip[:], den[:])
                state[(qb, j, "r")] = recip

            def emit_norm_b(qb, j):
                recip = state.pop((qb, j, "r"))
                c_a, c_b = state.pop((qb, j))
                q0 = qb * QB
                bc = ps_c.tile([P, QB], F32, tag="c")
                nc.tensor.matmul(
                    bc[:], lhsT=selp_sb[:], rhs=recip[:], start=True, stop=True
                )
                nc.vector.tensor_mul(
                    out=cn_sb[:, j, q0 : q0 + QB],
                    in0=c_sb[:, j, q0 : q0 + QB],
                    in1=bc[:],
                )

            # unit order: pair-1 enters after 3 pair-0 units so its Q/K
            # projections (fillers in units 1-2) are done; out-proj for a
            # query block fills a later unit once both pairs are normalized.
            units = [
                (0, 0), (1, 0), (2, 0), (0, 1),
                (3, 0), (1, 1), (2, 1), (3, 1),
            ]
            cols = [mk_proj_col(wq_sb, q_sb, c) for c in range(NQB)] + [
                mk_proj_col(wk_sb, k_sb, c) for c in range(NQB)
            ]
            fillers_by_idx = {
                1: cols[0:4],
                2: cols[4:8],
                5: [mk_outproj_mt(0, mt) for mt in range(DC)],
                7: [mk_outproj_mt(1, mt) for mt in range(DC)],
            }
            strides = {1: 4, 2: 4, 5: 2, 7: 2}
            for idx, (qb, j) in enumerate(units):
                emit_attention(
                    qb, j, fillers_by_idx.get(idx, ()), strides.get(idx, 4)
                )
                emit_norm_a(qb, j)
                if idx >= 1:
                    emit_norm_b(*units[idx - 1])
            emit_norm_b(*units[-1])
            tail_blocks = [(qb, mt) for qb in (2, 3) for mt in range(DC)]
            for i, (qb, mt) in enumerate(tail_blocks):
                mk_outproj_mt(qb, mt, pool=ps_c if i % 2 else None)()

    if not nc.is_finalized():
        nc.finalize()
    return nc


_NC_CACHE = {}


def _get_nc():
    if "nc" not in _NC_CACHE:
        _NC_CACHE["nc"] = build()
    return _NC_CACHE["nc"]


def _run(x, Wq, Wk, Wv, Wo, trace=False):
    """x: (B, D, L) f32; W*: (D, D) f32. Returns (out, BassKernelResults)."""
    nc = _get_nc()
    bf = ml_dtypes.bfloat16
    xb = np.ascontiguousarray(x).astype(bf)                 # (B, D, L)
    wqt = np.ascontiguousarray(np.asarray(Wq, np.float32).T).astype(bf)
    wkt = np.ascontiguousarray(np.asarray(Wk, np.float32).T).astype(bf)
    wvt = np.ascontiguousarray(np.asarray(Wv, np.float32).T).astype(bf)
    wot = np.ascontiguousarray(np.asarray(Wo, np.float32).T).astype(bf)

    selp = np.zeros((2, P), np.float32)
    selp[0, 0:DH] = 1.0
    selp[1, DH:P] = 1.0
    selp = selp.astype(bf)

    in_maps = []
    for c in range(8):
        b = c // 4
        g = c % 4
        r0 = g * NPAIR * P
        in_maps.append(
            {
                "x": xb[b],
                "wq": np.ascontiguousarray(wqt[:, r0 : r0 + NPAIR * P]),
                "wk": np.ascontiguousarray(wkt[:, r0 : r0 + NPAIR * P]),
                "wv": np.ascontiguousarray(wvt[:, r0 : r0 + NPAIR * P]),
                "wo": np.ascontiguousarray(wot[r0 : r0 + NPAIR * P, :]),
                "selp": selp,
            }
        )
    res = run_bass_kernel_spmd(nc, in_maps, core_ids=list(range(8)), trace=trace)
    out = np.zeros((B, D, L), np.float32)
    for c in range(8):
        b = c // 4
        out[b] += res.results[c]["out"].astype(np.float32)
    return out, res


def kernel(x, mask, Wq, Wk, Wv, Wo):
    # mask is all-ones by construction (fill: ones) -- softmax over all keys.
    out, _ = _run(x, Wq, Wk, Wv, Wo, trace=False)
    return out


# revision 15
# speedup vs baseline: 1.4001x; 1.0291x over previous
"""Multi-head attention (B=2, D=1024, L=2048, H=16) on 8 TRN2 NeuronCores.

Sharding (per spec hint): tensor-parallel over heads x data-parallel over
batch.  Core c handles batch c//4 and head group g=c%4 (4 heads as 2
"pairs" of 2 heads stacked on 64-partition halves).  Host sums the 4
bf16 partial outputs per batch (row-parallel W_O) in f32 -- no on-device
collective.

Per core:
  - Q/K projected into pair layout (head dims on partitions); V projected
    directly into transposed layout VT (Lk x dh) with a ones column per
    head so A@V also emits the softmax denominator.
  - Scores ST[k,q]: the two heads' K=64 matmuls are issued back-to-back
    at partition bases 0/64; they run concurrently on disjoint PE row
    groups (~2x, verified in trace).
  - exp runs on the scalar/ACT engine (the only exp engine): 128 x
    (128,1024) activations ~= 142us is the kernel's pacing floor.  All
    other PE work (pair-1 Q/K projections, output projection) is emitted
    as fillers INSIDE attention units so it executes in the PE's
    ACT-bound gaps instead of serializing.
  - Normalization per (pair, qb): denominator rows -> DVE reciprocal ->
    K=2 selector matmul broadcast -> one multiply.  The matmul+multiply
    (norm_b) is emitted one unit later so the PE queue never waits on
    the (slow, ~3.3us) DVE reciprocal.

All matmuls bf16 (f32 PSUM accumulate); softmax stats f32.
"""

import sys
import types

import numpy as np
import ml_dtypes


def _install_axon_hooks_shim():
    try:
        import antenv.axon_hooks  # noqa: F401
        return
    except ImportError:
        pass
    try:
        import antenv
    except ImportError:
        return
    mod = types.ModuleType("antenv.axon_hooks")
    mod._hook = None
    mod.set_axon_ntff_profile_hook = lambda h: setattr(mod, "_hook", h)
    mod.get_axon_ntff_profile_hook = lambda: mod._hook
    sys.modules["antenv.axon_hooks"] = mod
    antenv.axon_hooks = mod
    try:
        from trn_agent_boot.trn_boot import _ntff_profile_via_ctypes

        h = _ntff_profile_via_ctypes("/opt/axon/libaxon_pjrt.so")
        if h is not None:
            mod._hook = h
    except Exception:
        pass


_install_axon_hooks_shim()

import concourse.bass as bass  # noqa: E402
import concourse.mybir as mybir  # noqa: E402
import concourse.tile as tile  # noqa: E402
from concourse import bacc  # noqa: E402
from concourse.bass_utils import run_bass_kernel_spmd  # noqa: E402

BF16 = mybir.dt.bfloat16
F32 = mybir.dt.float32
AF = mybir.ActivationFunctionType

B, D, L, H = 2, 1024, 2048, 16
DH = D // H            # 64
P = 128
SCALE = 1.0 / np.sqrt(np.float32(DH))

DC = D // P            # 8 contraction chunks over D
LT = L // P            # 16 Lk tiles
HV = DH + 1            # V^T per-head width incl. ones column
NPAIR = 2              # head pairs per core (4 heads)
NQB = 4                # query blocks of 512
QB = L // NQB          # 512


def build():
    nc = bacc.Bacc(None, target_bir_lowering=False, debug=False)

    x = nc.dram_tensor("x", [D, L], BF16, kind="ExternalInput")
    wq = nc.dram_tensor("wq", [D, NPAIR * P], BF16, kind="ExternalInput")
    wk = nc.dram_tensor("wk", [D, NPAIR * P], BF16, kind="ExternalInput")
    wv = nc.dram_tensor("wv", [D, NPAIR * P], BF16, kind="ExternalInput")
    wo = nc.dram_tensor("wo", [NPAIR * P, D], BF16, kind="ExternalInput")
    selp = nc.dram_tensor("selp", [2, P], BF16, kind="ExternalInput")
    out = nc.dram_tensor("out", [D, L], BF16, kind="ExternalOutput")

    xr = x[:].rearrange("(o p) l -> p o l", p=P)          # (128, 8, 2048)
    wqr = wq[:].rearrange("(o p) m -> p o m", p=P)        # (128, 8, 256)
    wkr = wk[:].rearrange("(o p) m -> p o m", p=P)
    wvr = wv[:].rearrange("(o p) m -> p o m", p=P)
    wor = wo[:].rearrange("(j p) o -> p j o", p=P)        # (128, 2, 1024)
    outr = out[:].rearrange("(o p) l -> p o l", p=P)      # (128, 8, 2048)

    with tile.TileContext(nc) as tc:
        with (
            tc.tile_pool(name="consts", bufs=1) as consts,
            tc.tile_pool(name="res", bufs=1) as res,
            tc.tile_pool(name="exp", bufs=3) as epool,
            tc.tile_pool(name="norm", bufs=2) as npool,
            tc.tile_pool(name="outp", bufs=3) as opool,
            tc.tile_pool(name="ps_s", bufs=2, space="PSUM") as ps_s,
            tc.tile_pool(name="ps_c", bufs=3, space="PSUM") as ps_c,
            tc.tile_pool(name="ps_f", bufs=1, space="PSUM") as ps_f,
        ):
            selp_sb = consts.tile([2, P], BF16)
            nc.sync.dma_start(out=selp_sb[:], in_=selp[:])
            wq_sb = res.tile([P, DC, NPAIR * P], BF16)
            nc.sync.dma_start(out=wq_sb[:], in_=wqr)
            wk_sb = res.tile([P, DC, NPAIR * P], BF16)

            # x fanned out over four DMA queues so the last chunk lands
            # early; the startup-critical wq/wk share the sync queue head.
            xb = res.tile([P, DC, L], BF16)
            for kt in (0, 1, 2, 3, 4, 5):
                eng = (nc.gpsimd, nc.scalar)[kt % 2]
                eng.dma_start(out=xb[:, kt, :], in_=xr[:, kt, :])
            nc.sync.dma_start(out=wk_sb[:], in_=wkr)
            for kt in (6, 7):
                nc.sync.dma_start(out=xb[:, kt, :], in_=xr[:, kt, :])
            wv_sb = res.tile([P, DC, NPAIR * P], BF16)
            nc.gpsimd.dma_start(out=wv_sb[:], in_=wvr)
            wo_sb = res.tile([P, NPAIR, D], BF16)
            nc.gpsimd.dma_start(out=wo_sb[:], in_=wor)

            q_sb = res.tile([P, NPAIR, L], BF16)
            k_sb = res.tile([P, NPAIR, L], BF16)
            vt_sb = res.tile([P, LT, 2 * NPAIR * HV], BF16)
            vt4 = vt_sb[:].rearrange("p l (h e) -> p l h e", e=HV)
            nc.vector.memset(vt4[:, :, :, DH : DH + 1], 1.0)
            c_sb = res.tile([P, NPAIR, L], F32)     # unnormalized C
            cn_sb = res.tile([P, NPAIR, L], BF16)   # normalized C

            # ---- upfront: pair-0 Q/K projections (kt-outer, ldweights
            # reuse across the 4 q-columns), full V projection ----
            # kt consumption order roughly matching 3-queue DMA arrival
            KT_ORDER = (0, 1, 2, 3, 4, 5, 6, 7)

            def proj_pair(w_sb, dst, j):
                psA = ps_s.tile([P, 2 * QB], F32, tag="s")
                psB = ps_s.tile([P, 2 * QB], F32, tag="s")
                for ki, kt in enumerate(KT_ORDER):
                    lhsT = w_sb[:, kt, j * P : (j + 1) * P]
                    for half, ps in ((0, psA), (1, psB)):
                        for cb in range(2):
                            n0 = cb * QB
                            nc.tensor.matmul(
                                ps[:, n0 : n0 + QB],
                                lhsT=lhsT,
                                rhs=xb[:, kt, half * 1024 + n0 : half * 1024 + n0 + QB],
                                start=(ki == 0),
                                stop=(ki == DC - 1),
                            )
                nc.vector.tensor_copy(out=dst[:, j, 0:1024], in_=psA[:])
                nc.vector.tensor_copy(out=dst[:, j, 1024:2048], in_=psB[:])

            proj_pair(wq_sb, q_sb, 0)
            proj_pair(wk_sb, k_sb, 0)

            def emit_vtile(lt, pool, tag):
                psv = pool.tile([P, 2 * NPAIR * DH], F32, tag=tag)
                for kt in range(DC):
                    nc.tensor.matmul(
                        psv[:],
                        lhsT=xb[:, kt, lt * P : (lt + 1) * P],
                        rhs=wv_sb[:, kt, :],
                        start=(kt == 0),
                        stop=(kt == DC - 1),
                    )
                nc.vector.tensor_copy(
                    out=vt4[:, lt, :, 0:DH],
                    in_=psv[:].rearrange("p (h e) -> p h e", e=DH),
                )

            # first 4 V tiles upfront; the rest become unit-0 fillers
            for lt in range(4):
                emit_vtile(lt, ps_c, "c")

            # ---- filler generators (run inside attention units) ----
            def mk_proj_col(w_sb, dst, col):
                # one 512-wide column of the pair-1 Q or K projection
                def f():
                    pc = ps_f.tile([P, QB], F32, tag="f")
                    for kt in range(DC):
                        nc.tensor.matmul(
                            pc[:],
                            lhsT=w_sb[:, kt, P : 2 * P],
                            rhs=xb[:, kt, col * QB : (col + 1) * QB],
                            start=(kt == 0),
                            stop=(kt == DC - 1),
                        )
                    nc.vector.tensor_copy(
                        out=dst[:, 1, col * QB : (col + 1) * QB], in_=pc[:]
                    )
                return f

            def mk_outproj_mt(qb, mt, pool=None, ceng=None):
                # one 128-row block of the output projection for query block qb
                def f():
                    po = (pool or ps_f).tile(
                        [P, QB], F32, tag="f" if pool is None else "c"
                    )
                    q0 = qb * QB
                    for j in range(NPAIR):
                        nc.tensor.matmul(
                            po[:],
                            lhsT=wo_sb[:, j, mt * P : (mt + 1) * P],
                            rhs=cn_sb[:, j, q0 : q0 + QB],
                            start=(j == 0),
                            stop=(j == NPAIR - 1),
                        )
                    o_t = opool.tile([P, QB], BF16, tag="ot")
                    if ceng is nc.scalar:
                        nc.scalar.copy(o_t[:], po[:])
                    else:
                        nc.vector.tensor_copy(out=o_t[:], in_=po[:])
                    nc.sync.dma_start(out=outr[:, mt, q0 : q0 + QB], in_=o_t[:])
                return f

            # ---- attention units, software-pipelined across unit
            # boundaries: the next score pair is always emitted before the
            # current A@V so the ACT engine never drains its queue ----
            state = {}
            score_tiles = {}

            def emit_score(qb, j, t):
                q0 = qb * QB
                s = ps_s.tile([P, 2 * QB], F32, tag="s")
                nc.tensor.matmul(
                    s[:, 0:QB],
                    lhsT=k_sb[0:DH, j, t * P : (t + 1) * P],
                    rhs=q_sb[0:DH, j, q0 : q0 + QB],
                    start=True,
                    stop=True,
                )
                nc.tensor.matmul(
                    s[:, QB : 2 * QB],
                    lhsT=k_sb[DH:P, j, t * P : (t + 1) * P],
                    rhs=q_sb[DH:P, j, q0 : q0 + QB],
                    start=True,
                    stop=True,
                )
                score_tiles[(qb, j, t)] = s

            def emit_attention(qb, j, fillers=(), stride=4, next_first=None):
                c_a = ps_c.tile([HV, QB], F32, tag="c")
                c_b = ps_c.tile([HV, QB], F32, tag="c")
                fl = list(fillers)
                if (qb, j, 0) not in score_tiles:
                    emit_score(qb, j, 0)
                for t in range(LT):
                    s = score_tiles.pop((qb, j, t))
                    e = epool.tile([P, 2 * QB], BF16, tag="e")
                    nc.scalar.activation(e[:], s[:], AF.Exp, scale=float(SCALE))
                    if t < LT - 1:
                        emit_score(qb, j, t + 1)
                    elif next_first is not None:
                        emit_score(*next_first, 0)
                    nc.tensor.matmul(
                        c_a[:],
                        lhsT=vt4[:, t, 2 * j, :],
                        rhs=e[:, 0:QB],
                        start=(t == 0),
                        stop=(t == LT - 1),
                    )
                    nc.tensor.matmul(
                        c_b[:],
                        lhsT=vt4[:, t, 2 * j + 1, :],
                        rhs=e[:, QB : 2 * QB],
                        start=(t == 0),
                        stop=(t == LT - 1),
                    )
                    if fl and t % stride == stride - 1:
                        fl.pop(0)()
                for f in fl:
                    f()
                state[(qb, j)] = (c_a, c_b)

            def emit_norm_a(qb, j):
                # DVE-only: drain C (frees the c ring first), stage
                # denominators, reciprocal.
                c_a, c_b = state[(qb, j)]
                q0 = qb * QB
                nc.vector.tensor_copy(
                    out=c_sb[0:DH, j, q0 : q0 + QB], in_=c_a[0:DH, :]
                )
                nc.vector.tensor_copy(
                    out=c_sb[DH:P, j, q0 : q0 + QB], in_=c_b[0:DH, :]
                )
                den = npool.tile([2, QB], F32, tag="den")
                nc.vector.tensor_copy(out=den[0:1, :], in_=c_a[DH : DH + 1, :])
                stage = npool.tile([1, QB], F32, tag="stg")
                nc.vector.tensor_copy(out=stage[:], in_=c_b[DH : DH + 1, :])
                nc.sync.dma_start(out=den[1:2, :], in_=stage[:])
                recip = npool.tile([2, QB], BF16, tag="rcp")
                with nc.allow_low_precision(reason="bf16 1/den ok for 2e-2 tol"):
                    nc.vector.reciprocal(recip[:], den[:])
                state[(qb, j, "r")] = recip

            def emit_norm_b(qb, j):
                recip = state.pop((qb, j, "r"))
                c_a, c_b = state.pop((qb, j))
                q0 = qb * QB
                bc = ps_c.tile([P, QB], F32, tag="c")
                nc.tensor.matmul(
                    bc[:], lhsT=selp_sb[:], rhs=recip[:], start=True, stop=True
                )
                nc.vector.tensor_mul(
                    out=cn_sb[:, j, q0 : q0 + QB],
                    in0=c_sb[:, j, q0 : q0 + QB],
                    in1=bc[:],
                )

            # unit order: pair-1 enters after 3 pair-0 units so its Q/K
            # projections (fillers in units 1-2) are done; out-proj for a
            # query block fills a later unit once both pairs are normalized;
            # only qb3's out-proj remains for the tail.
            units = [
                (0, 0), (1, 0), (2, 0), (0, 1),
                (1, 1), (2, 1), (3, 0), (3, 1),
            ]
            cols = [mk_proj_col(wq_sb, q_sb, c) for c in range(NQB)] + [
                mk_proj_col(wk_sb, k_sb, c) for c in range(NQB)
            ]
            fillers_by_idx = {
                0: [(lambda lt: (lambda: emit_vtile(lt, ps_f, "f")))(lt)
                    for lt in range(4, LT)],
                1: cols[0:4],
                2: cols[4:8],
                5: [mk_outproj_mt(0, mt) for mt in range(DC)],
                6: [mk_outproj_mt(1, mt) for mt in range(DC)],
                7: [mk_outproj_mt(2, mt) for mt in range(DC)],
            }
            strides = {0: 1, 1: 4, 2: 4, 5: 2, 6: 2, 7: 2}
            for idx, (qb, j) in enumerate(units):
                emit_attention(
                    qb, j,
                    fillers_by_idx.get(idx, ()),
                    strides.get(idx, 4),
                    next_first=units[idx + 1] if idx + 1 < len(units) else None,
                )
                emit_norm_a(qb, j)
                if idx >= 1:
                    emit_norm_b(*units[idx - 1])
            emit_norm_b(*units[-1])
            for mt in range(DC):
                mk_outproj_mt(
                    3, mt, pool=ps_c if mt % 2 else None,
                    ceng=nc.scalar if mt % 2 else nc.vector,
                )()

    if not nc.is_finalized():
        nc.finalize()
    return nc


_NC_CACHE = {}


def _get_nc():
    if "nc" not in _NC_CACHE:
        _NC_CACHE["nc"] = build()
    return _NC_CACHE["nc"]


def _run(x, Wq, Wk, Wv, Wo, trace=False):
    """x: (B, D, L) f32; W*: (D, D) f32. Returns (out, BassKernelResults)."""
    nc = _get_nc()
    bf = ml_dtypes.bfloat16
    xb = np.ascontiguousarray(x).astype(bf)                 # (B, D, L)
    wqt = np.ascontiguousarray(np.asarray(Wq, np.float32).T).astype(bf)
    wkt = np.ascontiguousarray(np.asarray(Wk, np.float32).T).astype(bf)
    wvt = np.ascontiguousarray(np.asarray(Wv, np.float32).T).astype(bf)
    wot = np.ascontiguousarray(np.asarray(Wo, np.float32).T).astype(bf)

    selp = np.zeros((2, P), np.float32)
    selp[0, 0:DH] = 1.0
    selp[1, DH:P] = 1.0
    selp = selp.astype(bf)

    in_maps = []
    for c in range(8):
        b = c // 4
        g = c % 4
        r0 = g * NPAIR * P
        in_maps.append(
            {
                "x": xb[b],
                "wq": np.ascontiguousarray(wqt[:, r0 : r0 + NPAIR * P]),
                "wk": np.ascontiguousarray(wkt[:, r0 : r0 + NPAIR * P]),
                "wv": np.ascontiguousarray(wvt[:, r0 : r0 + NPAIR * P]),
                "wo": np.ascontiguousarray(wot[r0 : r0 + NPAIR * P, :]),
                "selp": selp,
            }
        )
    res = run_bass_kernel_spmd(nc, in_maps, core_ids=list(range(8)), trace=trace)
    out = np.zeros((B, D, L), np.float32)
    for c in range(8):
        b = c // 4
        out[b] += res.results[c]["out"].astype(np.float32)
    return out, res


def kernel(x, mask, Wq, Wk, Wv, Wo):
    # mask is all-ones by construction (fill: ones) -- softmax over all keys.
    out, _ = _run(x, Wq, Wk, Wv, Wo, trace=False)
    return out


# revision 20
# speedup vs baseline: 1.4265x; 1.0189x over previous
"""Multi-head attention (B=2, D=1024, L=2048, H=16) on 8 TRN2 NeuronCores.

Sharding (per spec hint): tensor-parallel over heads x data-parallel over
batch.  Core c handles batch c//4 and head group g=c%4 (4 heads as 2
"pairs" of 2 heads stacked on 64-partition halves).  Host sums the 4
bf16 partial outputs per batch (row-parallel W_O) in f32 -- no on-device
collective.

Per core:
  - Q/K projected into pair layout (head dims on partitions); V projected
    directly into transposed layout VT (Lk x dh) with a ones column per
    head so A@V also emits the softmax denominator.
  - Scores ST[k,q]: the two heads' K=64 matmuls are issued back-to-back
    at partition bases 0/64; they run concurrently on disjoint PE row
    groups (~2x, verified in trace).
  - exp runs on the scalar/ACT engine (the only exp engine): 128 x
    (128,1024) activations ~= 142us is the kernel's pacing floor.  All
    other PE work (pair-1 Q/K projections, output projection) is emitted
    as fillers INSIDE attention units so it executes in the PE's
    ACT-bound gaps instead of serializing.
  - Normalization per (pair, qb): denominator rows -> DVE reciprocal ->
    K=2 selector matmul broadcast -> one multiply.  The matmul+multiply
    (norm_b) is emitted one unit later so the PE queue never waits on
    the (slow, ~3.3us) DVE reciprocal.

All matmuls bf16 (f32 PSUM accumulate); softmax stats f32.
"""

import sys
import types

import numpy as np
import ml_dtypes


def _install_axon_hooks_shim():
    try:
        import antenv.axon_hooks  # noqa: F401
        return
    except ImportError:
        pass
    try:
        import antenv
    except ImportError:
        return
    mod = types.ModuleType("antenv.axon_hooks")
    mod._hook = None
    mod.set_axon_ntff_profile_hook = lambda h: setattr(mod, "_hook", h)
    mod.get_axon_ntff_profile_hook = lambda: mod._hook
    sys.modules["antenv.axon_hooks"] = mod
    antenv.axon_hooks = mod
    try:
        from trn_agent_boot.trn_boot import _ntff_profile_via_ctypes

        h = _ntff_profile_via_ctypes("/opt/axon/libaxon_pjrt.so")
        if h is not None:
            mod._hook = h
    except Exception:
        pass


_install_axon_hooks_shim()

import concourse.bass as bass  # noqa: E402
import concourse.mybir as mybir  # noqa: E402
import concourse.tile as tile  # noqa: E402
from concourse import bacc  # noqa: E402
from concourse.bass_utils import run_bass_kernel_spmd  # noqa: E402

BF16 = mybir.dt.bfloat16
F32 = mybir.dt.float32
AF = mybir.ActivationFunctionType

B, D, L, H = 2, 1024, 2048, 16
DH = D // H            # 64
P = 128
SCALE = 1.0 / np.sqrt(np.float32(DH))

DC = D // P            # 8 contraction chunks over D
LT = L // P            # 16 Lk tiles
HV = DH + 1            # V^T per-head width incl. ones column
NPAIR = 2              # head pairs per core (4 heads)
NQB = 4                # query blocks of 512
QB = L // NQB          # 512


def build():
    nc = bacc.Bacc(None, target_bir_lowering=False, debug=False)

    x = nc.dram_tensor("x", [D, L], BF16, kind="ExternalInput")
    wq = nc.dram_tensor("wq", [D, NPAIR * P], BF16, kind="ExternalInput")
    wk = nc.dram_tensor("wk", [D, NPAIR * P], BF16, kind="ExternalInput")
    wv = nc.dram_tensor("wv", [D, NPAIR * P], BF16, kind="ExternalInput")
    wo = nc.dram_tensor("wo", [NPAIR * P, D], BF16, kind="ExternalInput")
    selp = nc.dram_tensor("selp", [2, P], BF16, kind="ExternalInput")
    out = nc.dram_tensor("out", [D, L], BF16, kind="ExternalOutput")

    xr = x[:].rearrange("(o p) l -> p o l", p=P)          # (128, 8, 2048)
    wqr = wq[:].rearrange("(o p) m -> p o m", p=P)        # (128, 8, 256)
    wkr = wk[:].rearrange("(o p) m -> p o m", p=P)
    wvr = wv[:].rearrange("(o p) m -> p o m", p=P)
    wor = wo[:].rearrange("(j p) o -> p j o", p=P)        # (128, 2, 1024)
    outr = out[:].rearrange("(o p) l -> p o l", p=P)      # (128, 8, 2048)

    with tile.TileContext(nc) as tc:
        with (
            tc.tile_pool(name="consts", bufs=1) as consts,
            tc.tile_pool(name="res", bufs=1) as res,
            tc.tile_pool(name="exp", bufs=3) as epool,
            tc.tile_pool(name="norm", bufs=2) as npool,
            tc.tile_pool(name="outp", bufs=3) as opool,
            tc.tile_pool(name="ps_s", bufs=2, space="PSUM") as ps_s,
            tc.tile_pool(name="ps_c", bufs=3, space="PSUM") as ps_c,
            tc.tile_pool(name="ps_f", bufs=1, space="PSUM") as ps_f,
        ):
            # DMA need-order sequencing: the DMA engines round-robin among
            # ALL in-flight transfers, so if everything is enqueued at once
            # the startup-critical wq/x0 only land when ~everything lands
            # (~24us).  Instead each of the 3 queues carries its pieces in
            # consumption order so the first wave (wq, x0, x1) gets the
            # full bandwidth.
            selp_sb = consts.tile([2, P], BF16)
            wq_sb = res.tile([P, DC, NPAIR * P], BF16)
            wk_sb = res.tile([P, DC, NPAIR * P], BF16)
            wv_sb = res.tile([P, DC, NPAIR * P], BF16)
            wo_sb = res.tile([P, NPAIR, D], BF16)
            xb = res.tile([P, DC, L], BF16)

            nc.sync.dma_start(out=selp_sb[:], in_=selp[:])
            nc.sync.dma_start(out=wq_sb[:], in_=wqr)
            nc.gpsimd.dma_start(out=xb[:, 0, :], in_=xr[:, 0, :])
            nc.scalar.dma_start(out=xb[:, 1, :], in_=xr[:, 1, :])
            nc.sync.dma_start(out=xb[:, 2, :], in_=xr[:, 2, :])
            nc.gpsimd.dma_start(out=xb[:, 3, :], in_=xr[:, 3, :])
            nc.scalar.dma_start(out=xb[:, 4, :], in_=xr[:, 4, :])
            nc.sync.dma_start(out=wk_sb[:], in_=wkr)
            nc.gpsimd.dma_start(out=xb[:, 6, :], in_=xr[:, 6, :])
            nc.scalar.dma_start(out=xb[:, 7, :], in_=xr[:, 7, :])
            nc.sync.dma_start(out=xb[:, 5, :], in_=xr[:, 5, :])
            nc.gpsimd.dma_start(out=wv_sb[:], in_=wvr)
            nc.gpsimd.dma_start(out=wo_sb[:], in_=wor)

            q_sb = res.tile([P, NPAIR, L], BF16)
            k_sb = res.tile([P, NPAIR, L], BF16)
            vt_sb = res.tile([P, LT, 2 * NPAIR * HV], BF16)
            vt4 = vt_sb[:].rearrange("p l (h e) -> p l h e", e=HV)
            nc.vector.memset(vt4[:, :, :, DH : DH + 1], 1.0)
            c_sb = res.tile([P, NPAIR, L], F32)     # unnormalized C
            cn_sb = res.tile([P, NPAIR, L], BF16)   # normalized C

            # ---- upfront: pair-0 Q/K projections (kt-outer, ldweights
            # reuse across the 4 q-columns), full V projection ----
            # kt consumption order roughly matching 3-queue DMA arrival
            KT_ORDER = (0, 1, 2, 3, 4, 6, 7, 5)

            def proj_pair(w_sb, dst, j):
                psA = ps_s.tile([P, 2 * QB], F32, tag="s")
                psB = ps_s.tile([P, 2 * QB], F32, tag="s")
                for ki, kt in enumerate(KT_ORDER):
                    lhsT = w_sb[:, kt, j * P : (j + 1) * P]
                    for half, ps in ((0, psA), (1, psB)):
                        for cb in range(2):
                            n0 = cb * QB
                            nc.tensor.matmul(
                                ps[:, n0 : n0 + QB],
                                lhsT=lhsT,
                                rhs=xb[:, kt, half * 1024 + n0 : half * 1024 + n0 + QB],
                                start=(ki == 0),
                                stop=(ki == DC - 1),
                            )
                nc.vector.tensor_copy(out=dst[:, j, 0:1024], in_=psA[:])
                nc.vector.tensor_copy(out=dst[:, j, 1024:2048], in_=psB[:])

            proj_pair(wq_sb, q_sb, 0)

            # K pair-0: only columns 0:1024 upfront (scores t<8); the other
            # half runs as unit-0 fillers so attention starts sooner.
            psK = ps_s.tile([P, 2 * QB], F32, tag="s")
            for ki, kt in enumerate(KT_ORDER):
                for cb in range(2):
                    n0 = cb * QB
                    nc.tensor.matmul(
                        psK[:, n0 : n0 + QB],
                        lhsT=wk_sb[:, kt, 0:P],
                        rhs=xb[:, kt, n0 : n0 + QB],
                        start=(ki == 0),
                        stop=(ki == DC - 1),
                    )
            nc.vector.tensor_copy(out=k_sb[:, 0, 0:1024], in_=psK[:])

            def emit_vtile(lt, pool, tag):
                psv = pool.tile([P, 2 * NPAIR * DH], F32, tag=tag)
                for kt in range(DC):
                    nc.tensor.matmul(
                        psv[:],
                        lhsT=xb[:, kt, lt * P : (lt + 1) * P],
                        rhs=wv_sb[:, kt, :],
                        start=(kt == 0),
                        stop=(kt == DC - 1),
                    )
                nc.vector.tensor_copy(
                    out=vt4[:, lt, :, 0:DH],
                    in_=psv[:].rearrange("p (h e) -> p h e", e=DH),
                )

            # first 4 V tiles upfront; the rest become unit-0 fillers
            for lt in range(4):
                emit_vtile(lt, ps_c, "c")

            # ---- filler generators (run inside attention units) ----
            def mk_proj_col(w_sb, dst, col, j=1):
                # one 512-wide column of a Q or K projection for pair j
                def f():
                    pc = ps_f.tile([P, QB], F32, tag="f")
                    for ki, kt in enumerate(KT_ORDER):
                        nc.tensor.matmul(
                            pc[:],
                            lhsT=w_sb[:, kt, j * P : (j + 1) * P],
                            rhs=xb[:, kt, col * QB : (col + 1) * QB],
                            start=(ki == 0),
                            stop=(ki == DC - 1),
                        )
                    nc.vector.tensor_copy(
                        out=dst[:, j, col * QB : (col + 1) * QB], in_=pc[:]
                    )
                return f

            def mk_outproj_mt(qb, mt, pool=None, ceng=None):
                # one 128-row block of the output projection for query block qb
                def f():
                    po = (pool or ps_f).tile(
                        [P, QB], F32, tag="f" if pool is None else "c"
                    )
                    q0 = qb * QB
                    for j in range(NPAIR):
                        nc.tensor.matmul(
                            po[:],
                            lhsT=wo_sb[:, j, mt * P : (mt + 1) * P],
                            rhs=cn_sb[:, j, q0 : q0 + QB],
                            start=(j == 0),
                            stop=(j == NPAIR - 1),
                        )
                    o_t = opool.tile([P, QB], BF16, tag="ot")
                    if ceng is nc.scalar:
                        nc.scalar.copy(o_t[:], po[:])
                    else:
                        nc.vector.tensor_copy(out=o_t[:], in_=po[:])
                    nc.sync.dma_start(out=outr[:, mt, q0 : q0 + QB], in_=o_t[:])
                return f

            # ---- attention units, software-pipelined across unit
            # boundaries: the next score pair is always emitted before the
            # current A@V so the ACT engine never drains its queue ----
            state = {}
            score_tiles = {}

            def emit_score(qb, j, t):
                q0 = qb * QB
                s = ps_s.tile([P, 2 * QB], F32, tag="s")
                nc.tensor.matmul(
                    s[:, 0:QB],
                    lhsT=k_sb[0:DH, j, t * P : (t + 1) * P],
                    rhs=q_sb[0:DH, j, q0 : q0 + QB],
                    start=True,
                    stop=True,
                )
                nc.tensor.matmul(
                    s[:, QB : 2 * QB],
                    lhsT=k_sb[DH:P, j, t * P : (t + 1) * P],
                    rhs=q_sb[DH:P, j, q0 : q0 + QB],
                    start=True,
                    stop=True,
                )
                score_tiles[(qb, j, t)] = s

            def emit_attention(qb, j, fillers=(), stride=4, next_first=None):
                c_a = ps_c.tile([HV, QB], F32, tag="c")
                c_b = ps_c.tile([HV, QB], F32, tag="c")
                fl = list(fillers)
                if (qb, j, 0) not in score_tiles:
                    emit_score(qb, j, 0)
                for t in range(LT):
                    s = score_tiles.pop((qb, j, t))
                    e = epool.tile([P, 2 * QB], BF16, tag="e")
                    nc.scalar.activation(e[:], s[:], AF.Exp, scale=float(SCALE))
                    if t < LT - 1:
                        emit_score(qb, j, t + 1)
                    elif next_first is not None:
                        emit_score(*next_first, 0)
                    nc.tensor.matmul(
                        c_a[:],
                        lhsT=vt4[:, t, 2 * j, :],
                        rhs=e[:, 0:QB],
                        start=(t == 0),
                        stop=(t == LT - 1),
                    )
                    nc.tensor.matmul(
                        c_b[:],
                        lhsT=vt4[:, t, 2 * j + 1, :],
                        rhs=e[:, QB : 2 * QB],
                        start=(t == 0),
                        stop=(t == LT - 1),
                    )
                    if fl and t % stride == stride - 1:
                        fl.pop(0)()
                for f in fl:
                    f()
                state[(qb, j)] = (c_a, c_b)

            def emit_norm_a(qb, j):
                # DVE-only: drain C (frees the c ring first), stage
                # denominators, reciprocal.
                c_a, c_b = state[(qb, j)]
                q0 = qb * QB
                nc.vector.tensor_copy(
                    out=c_sb[0:DH, j, q0 : q0 + QB], in_=c_a[0:DH, :]
                )
                nc.vector.tensor_copy(
                    out=c_sb[DH:P, j, q0 : q0 + QB], in_=c_b[0:DH, :]
                )
                den = npool.tile([2, QB], F32, tag="den")
                nc.vector.tensor_copy(out=den[0:1, :], in_=c_a[DH : DH + 1, :])
                stage = npool.tile([1, QB], F32, tag="stg")
                nc.vector.tensor_copy(out=stage[:], in_=c_b[DH : DH + 1, :])
                nc.sync.dma_start(out=den[1:2, :], in_=stage[:])
                recip = npool.tile([2, QB], BF16, tag="rcp")
                with nc.allow_low_precision(reason="bf16 1/den ok for 2e-2 tol"):
                    nc.vector.reciprocal(recip[:], den[:])
                state[(qb, j, "r")] = recip

            def emit_norm_b(qb, j):
                recip = state.pop((qb, j, "r"))
                c_a, c_b = state.pop((qb, j))
                q0 = qb * QB
                bc = ps_c.tile([P, QB], F32, tag="c")
                nc.tensor.matmul(
                    bc[:], lhsT=selp_sb[:], rhs=recip[:], start=True, stop=True
                )
                nc.vector.tensor_mul(
                    out=cn_sb[:, j, q0 : q0 + QB],
                    in0=c_sb[:, j, q0 : q0 + QB],
                    in1=bc[:],
                )

            # unit order: pair-1 enters after 3 pair-0 units so its Q/K
            # projections (fillers in units 1-2) are done; out-proj for a
            # query block fills a later unit once both pairs are normalized;
            # only qb3's out-proj remains for the tail.
            units = [
                (0, 0), (1, 0), (2, 0), (0, 1),
                (1, 1), (2, 1), (3, 0), (3, 1),
            ]
            cols = [mk_proj_col(wq_sb, q_sb, c) for c in range(NQB)] + [
                mk_proj_col(wk_sb, k_sb, c) for c in range(NQB)
            ]
            fillers_by_idx = {
                0: [mk_proj_col(wk_sb, k_sb, 2, j=0),
                    mk_proj_col(wk_sb, k_sb, 3, j=0)]
                   + [(lambda lt: (lambda: emit_vtile(lt, ps_f, "f")))(lt)
                      for lt in range(4, LT)],
                1: cols[0:4],
                2: cols[4:8],
                5: [mk_outproj_mt(0, mt) for mt in range(DC)],
                6: [mk_outproj_mt(1, mt) for mt in range(DC)],
                7: [mk_outproj_mt(2, mt) for mt in range(DC)],
            }
            strides = {0: 1, 1: 4, 2: 4, 5: 2, 6: 2, 7: 2}
            for idx, (qb, j) in enumerate(units):
                emit_attention(
                    qb, j,
                    fillers_by_idx.get(idx, ()),
                    strides.get(idx, 4),
                    next_first=units[idx + 1] if idx + 1 < len(units) else None,
                )
                # norm_b for the PREVIOUS unit goes into the DVE queue before
                # this unit's norm_a chain (whose reciprocal is slow), so the
                # next unit's out-proj fillers never wait on it.
                if idx >= 1:
                    emit_norm_b(*units[idx - 1])
                emit_norm_a(qb, j)
            emit_norm_b(*units[-1])
            for mt in range(DC):
                mk_outproj_mt(
                    3, mt, pool=ps_c if mt % 2 else None,
                    ceng=nc.scalar if mt % 2 else nc.vector,
                )()

    if not nc.is_finalized():
        nc.finalize()
    return nc


_NC_CACHE = {}


def _get_nc():
    if "nc" not in _NC_CACHE:
        _NC_CACHE["nc"] = build()
    return _NC_CACHE["nc"]


def _run(x, Wq, Wk, Wv, Wo, trace=False):
    """x: (B, D, L) f32; W*: (D, D) f32. Returns (out, BassKernelResults)."""
    nc = _get_nc()
    bf = ml_dtypes.bfloat16
    xb = np.ascontiguousarray(x).astype(bf)                 # (B, D, L)
    wqt = np.ascontiguousarray(np.asarray(Wq, np.float32).T).astype(bf)
    wkt = np.ascontiguousarray(np.asarray(Wk, np.float32).T).astype(bf)
    wvt = np.ascontiguousarray(np.asarray(Wv, np.float32).T).astype(bf)
    wot = np.ascontiguousarray(np.asarray(Wo, np.float32).T).astype(bf)

    selp = np.zeros((2, P), np.float32)
    selp[0, 0:DH] = 1.0
    selp[1, DH:P] = 1.0
    selp = selp.astype(bf)

    in_maps = []
    for c in range(8):
        b = c // 4
        g = c % 4
        r0 = g * NPAIR * P
        in_maps.append(
            {
                "x": xb[b],
                "wq": np.ascontiguousarray(wqt[:, r0 : r0 + NPAIR * P]),
                "wk": np.ascontiguousarray(wkt[:, r0 : r0 + NPAIR * P]),
                "wv": np.ascontiguousarray(wvt[:, r0 : r0 + NPAIR * P]),
                "wo": np.ascontiguousarray(wot[r0 : r0 + NPAIR * P, :]),
                "selp": selp,
            }
        )
    res = run_bass_kernel_spmd(nc, in_maps, core_ids=list(range(8)), trace=trace)
    out = np.zeros((B, D, L), np.float32)
    for c in range(8):
        b = c // 4
        out[b] += res.results[c]["out"].astype(np.float32)
    return out, res


def kernel(x, mask, Wq, Wk, Wv, Wo):
    # mask is all-ones by construction (fill: ones) -- softmax over all keys.
    out, _ = _run(x, Wq, Wk, Wv, Wo, trace=False)
    return out
